# revision 20
# baseline (speedup 1.0000x reference)
"""GAT node-level layer on 8 TRN2 NeuronCores.

Strategy: destination-sharded edge processing with bulk dma_gather.
 - Host: sort edges by dst, shard by dst range (6250 nodes/core),
   window = 128 consecutive dst, chunk = 128 edges.  Within each window
   edges are split into lo (src < 32768) and hi (src >= 32768) chunk
   groups because dma_gather indices are int16.  All structures padded
   to uniform CAPL/CAPH so one SPMD program serves all cores; padding
   slots gather row 0 (valid) and are killed by seg=-1 masks.
 - Device phase 1: z = h_shard @ W.T (bf16), q = z @ a_dst per node.
   AllGather z (bf16, Shared-HBM output); q packed hi/lo bf16 into a
   local q_pad[6250, 128] table (256B rows for dma_gather).
 - Device phase 2 per window: bulk-gather cap*128 z rows (256B each,
   ONE dma_gather per lo/hi group) + per-edge q rows from q_pad;
   s = <z_row, a_src> via per-chunk tensor_tensor_reduce;
   e = leaky_relu(s+q); exp without max-subtraction (shift cancels in
   softmax); selection matrix S[e,d] = (seg==d)*exp_e in one fused
   tensor_scalar per chunk; PE accumulates out[d,:] += S_c.T @ z_c and
   den[d] += S_c.T @ 1; normalize; write out.
No scatter; no inter-core traffic except the z AllGather.
"""

import sys

if "/opt/trn_rl_repo" not in sys.path:
    sys.path.insert(0, "/opt/trn_rl_repo")

from contextlib import ExitStack

import numpy as np

from concourse import bacc, bass, mybir, tile
from concourse.masks import make_identity

N_NODES = 50000
N_EDGES = 800000
D_IN = 256
D_OUT = 128
CORES = 8
P = 128
SPLIT = 32768  # int16 idx limit for dma_gather

F32 = mybir.dt.float32
BF16 = mybir.dt.bfloat16
I16 = mybir.dt.int16

_PROGRAM_CACHE: dict = {}
STRIP_LEVEL = 0  # debug: 3=gathers only, 2=+logits, 1=+matmuls, 0=full
LIMIT_WINDOWS = 0  # debug: >0 limits phase-2 window count


# ---------------------------------------------------------------- host prep
def preprocess_indices(src, dst, n_nodes=N_NODES, cores=CORES):
    """Sort edges by dst, shard by dst range, build per-core padded index
    arrays for dma_gather (int16, [16,S]-wrapped, replicated to 128
    partitions).  Returns (capl, caph, per-core dict)."""
    shard = n_nodes // cores
    wpc = (shard + P - 1) // P
    src = np.asarray(src).astype(np.int64)
    dst = np.asarray(dst).astype(np.int64)

    order = np.argsort(dst, kind="stable")
    ds = dst[order]
    ss = src[order]
    bounds = np.searchsorted(ds, np.arange(cores + 1) * shard)

    # per (core, window): lo/hi edge lists
    per_core = []
    capl = caph = 1
    for c in range(cores):
        lo_, hi_ = int(bounds[c]), int(bounds[c + 1])
        dloc = ds[lo_:hi_] - c * shard
        s_c = ss[lo_:hi_]
        w = dloc >> 7
        is_hi = s_c >= SPLIT
        nlo = np.bincount(w[~is_hi], minlength=wpc)
        nhi = np.bincount(w[is_hi], minlength=wpc)
        capl = max(capl, int((nlo.max() + P - 1) // P))
        caph = max(caph, int((nhi.max() + P - 1) // P))
        per_core.append((dloc, s_c, w, is_hi))

    cap = capl + caph
    arrs = []
    for c in range(cores):
        dloc, s_c, w, is_hi = per_core[c]
        # slot within window: lo edges first (slots [0, capl*128)),
        # hi edges at slots [capl*128, cap*128)
        slot = np.zeros(len(dloc), np.int64)
        for grp, base, width in ((~is_hi, 0, capl), (is_hi, capl * P, caph)):
            sel = np.flatnonzero(grp)
            if len(sel) == 0:
                continue
            ww = w[sel]
            o = np.argsort(ww, kind="stable")
            sel = sel[o]
            ww = w[sel]
            cnt = np.bincount(ww, minlength=wpc)
            st = np.zeros(wpc, np.int64)
            st[1:] = np.cumsum(cnt)[:-1]
            pos = np.arange(len(sel), dtype=np.int64) - st[ww]
            slot[sel] = base + pos

        chunk = slot >> 7
        epos = slot & 127

        # z gather indices, flat j = chunk_local*128 + epos, wrapped [16, S]
        zi = np.zeros((wpc, cap * P), np.int16)
        flat = w * (cap * P) + chunk * P + epos
        zi_flat = np.zeros(wpc * cap * P, np.int16)
        zval = np.where(is_hi, s_c - SPLIT, s_c).astype(np.int16)
        zi_flat[flat] = zval
        zi = zi_flat.reshape(wpc, cap, P)

        qi_flat = np.zeros(wpc * cap * P, np.int16)
        qi_flat[flat] = dloc.astype(np.int16)
        qi = qi_flat.reshape(wpc, cap, P)

        seg = np.full((P, wpc * cap), -1.0, np.float32)
        seg[epos, w * cap + chunk] = (dloc & 127).astype(np.float32)

        def wrap16(blocks):
            # blocks [wpc, cap, 128] flat j=c*128+p per window ->
            # [16, wpc*cap*8] with j at [j%16, j//16], replicated to 128
            b = blocks.reshape(wpc, cap * P)
            out = np.zeros((16, wpc, cap * P // 16), np.int16)
            j = np.arange(cap * P)
            out[j % 16, :, j // 16] = b.T[j]
            out = out.reshape(16, wpc * cap * P // 16)
            return np.ascontiguousarray(np.tile(out, (8, 1)))

        arrs.append(
            {
                "zi16": wrap16(zi),
                "qi16": wrap16(qi),
                "seg_all": np.ascontiguousarray(seg),
            }
        )
    return capl, caph, arrs


# ---------------------------------------------------------------- program
def build_program(capl, caph, n_nodes=N_NODES, d_in=D_IN, d_out=D_OUT, cores=CORES):
    shard = n_nodes // cores
    wpc = (shard + P - 1) // P
    kc_n = d_in // P
    cap = capl + caph
    nlo, nhi, nq = capl * P, caph * P, cap * P
    wcol = cap * P // 16  # idx16 columns per window

    nc = bacc.Bacc(None, target_bir_lowering=False, debug=False)

    h_t = nc.dram_tensor("h_t", [d_in, shard], F32, kind="ExternalInput")
    w_d = nc.dram_tensor("W", [d_out, d_in], F32, kind="ExternalInput")
    a_d = nc.dram_tensor("a", [2 * d_out, 1], F32, kind="ExternalInput")
    arep_d = nc.dram_tensor("arep", [P, P], F32, kind="ExternalInput")
    zi_d = nc.dram_tensor("zi16", [P, wpc * wcol], I16, kind="ExternalInput")
    qi_d = nc.dram_tensor("qi16", [P, wpc * wcol], I16, kind="ExternalInput")
    seg_d = nc.dram_tensor("seg_all", [P, wpc * cap], F32, kind="ExternalInput")
    out_d = nc.dram_tensor("out", [shard, d_out], F32, kind="ExternalOutput")

    rg = [list(range(cores))]

    with tile.TileContext(nc) as tc:
        with ExitStack() as ctx:
            dram = ctx.enter_context(tc.tile_pool(name="dram", bufs=1, space="DRAM"))
            z_bounce = dram.tile([shard, d_out], BF16)
            z_full = dram.tile([n_nodes, d_out], BF16)
            q_pad = dram.tile([shard, P], BF16)

            const = ctx.enter_context(tc.tile_pool(name="const", bufs=1))

            # ---- constants
            identity = const.tile([P, P], F32)
            make_identity(nc, identity[:])
            iota_i = const.tile([P, P], mybir.dt.int32)
            nc.gpsimd.iota(iota_i[:], pattern=[[1, P]], base=0, channel_multiplier=0)
            iota_bf = const.tile([P, P], BF16)
            nc.vector.tensor_copy(iota_bf[:], iota_i[:])
            ones_col = const.tile([P, 1], BF16)
            nc.gpsimd.memset(ones_col[:], 1.0)

            # edge index arrays (start loading immediately, HWDGE)
            zi_sb = const.tile([P, wpc * wcol], I16)
            qi_sb = const.tile([P, wpc * wcol], I16)
            seg_sb = const.tile([P, wpc * cap], F32)
            nc.sync.dma_start(out=zi_sb[:], in_=zi_d[:, :])
            nc.scalar.dma_start(out=qi_sb[:], in_=qi_d[:, :])
            nc.sync.dma_start(out=seg_sb[:], in_=seg_d[:, :])

            w_sb = const.tile([P, d_in], F32)
            nc.sync.dma_start(out=w_sb[:], in_=w_d[:, :])
            a_sb = const.tile([P, 2], F32)
            nc.sync.dma_start(out=a_sb[:, 0:1], in_=a_d[0:P, :])
            nc.sync.dma_start(out=a_sb[:, 1:2], in_=a_d[P : 2 * P, :])

            # arep[p, f] = a_src[f]  (host-replicated input)
            ctx1 = ctx.enter_context(ExitStack())
            psum = ctx1.enter_context(tc.tile_pool(name="psum", bufs=1, space="PSUM"))
            psum_pz = ctx1.enter_context(
                tc.tile_pool(name="psum_pz", bufs=2, space="PSUM")
            )
            arep_f = const.tile([P, P], F32)
            nc.sync.dma_start(out=arep_f[:], in_=arep_d[:, :])
            arep3 = const.tile([P, 1, P], BF16)
            nc.vector.tensor_copy(arep3[:, 0, :], arep_f[:])

            # waug[kc] = [W.T chunk | v_dst chunk]  (bf16)
            waug = const.tile([P, kc_n, d_out + 1], BF16)
            for kc in range(kc_n):
                ksl = slice(kc * P, (kc + 1) * P)
                pt = psum.tile([P, P], F32, tag="pt")
                nc.tensor.transpose(pt[:], w_sb[:, ksl], identity[:])
                nc.vector.tensor_copy(waug[:, kc, 0:d_out], pt[:])
                pv = psum.tile([P, 1], F32, tag="pv")
                nc.tensor.matmul(
                    out=pv[:, 0:1], lhsT=w_sb[:, ksl], rhs=a_sb[:, 1:2],
                    start=True, stop=True,
                )
                nc.vector.tensor_copy(waug[:, kc, d_out : d_out + 1], pv[:])

            # ---- phase 1: z = h @ W.T, q = z @ a_dst
            h_sb = const.tile([P, kc_n, shard], BF16)
            for kc in range(kc_n):
                nc.gpsimd.dma_start(
                    out=h_sb[:, kc, :], in_=h_t[kc * P : (kc + 1) * P, :]
                )

            zq = ctx.enter_context(tc.tile_pool(name="zq", bufs=3))
            for nt in range(wpc):
                n0 = nt * P
                rows = min(P, shard - n0)
                pz = psum_pz.tile([P, d_out + 1], F32, tag="pz")
                for kc in range(kc_n):
                    nc.tensor.matmul(
                        out=pz[0:rows, :],
                        lhsT=h_sb[:, kc, n0 : n0 + rows],
                        rhs=waug[:, kc, :],
                        start=(kc == 0),
                        stop=(kc == kc_n - 1),
                    )
                zt = zq.tile([P, d_out], BF16, tag="zt")
                nc.vector.tensor_copy(zt[0:rows, :], pz[0:rows, 0:d_out])
                qhl = zq.tile([P, 2], BF16, tag="qhl")
                nc.vector.tensor_copy(qhl[0:rows, 0:1], pz[0:rows, d_out : d_out + 1])
                nc.vector.tensor_tensor(
                    out=qhl[0:rows, 1:2],
                    in0=pz[0:rows, d_out : d_out + 1],
                    in1=qhl[0:rows, 0:1],
                    op=mybir.AluOpType.subtract,
                )
                if nt % 2 == 0:
                    nc.sync.dma_start(
                        out=z_bounce[n0 : n0 + rows, :], in_=zt[0:rows, :]
                    )
                else:
                    nc.scalar.dma_start(
                        out=z_bounce[n0 : n0 + rows, :], in_=zt[0:rows, :]
                    )
                nc.sync.dma_start(
                    out=q_pad[n0 : n0 + rows, 0:2], in_=qhl[0:rows, :]
                )

            ctx1.close()

            nc.gpsimd.collective_compute(
                "AllGather",
                mybir.AluOpType.bypass,
                replica_groups=rg,
                ins=[z_bounce[:, :]],
                outs=[z_full[:, :]],
            )

            # ---- phase 2: per dst-window edge processing
            gp = ctx.enter_context(tc.tile_pool(name="gp", bufs=3))
            gq = ctx.enter_context(tc.tile_pool(name="gq", bufs=3))
            wp = ctx.enter_context(tc.tile_pool(name="wp", bufs=3))
            sp = ctx.enter_context(tc.tile_pool(name="sp", bufs=4))
            op = ctx.enter_context(tc.tile_pool(name="op", bufs=3))
            psum2 = ctx.enter_context(
                tc.tile_pool(name="psum2", bufs=2, space="PSUM")
            )

            nwin = 0 if LIMIT_WINDOWS < 0 else (LIMIT_WINDOWS if LIMIT_WINDOWS > 0 else wpc)
            for w in range(nwin):
                n0 = w * P
                rows = min(P, shard - n0)
                c0 = w * cap
                i0 = w * wcol

                g = gp.tile([P, cap, d_out], BF16, tag="g")
                if STRIP_LEVEL == 5:
                    nc.gpsimd.memset(g[:], 0.0)
                if STRIP_LEVEL != 5:
                    nc.gpsimd.dma_gather(
                    g[:, 0:capl, :], z_full[0:SPLIT, :],
                    zi_sb[:, i0 : i0 + nlo // 16], nlo, nlo, d_out,
                        single_packet=False,
                    )
                    nc.gpsimd.dma_gather(
                        g[:, capl:cap, :], z_full[SPLIT:n_nodes, :],
                        zi_sb[:, i0 + nlo // 16 : i0 + wcol], nhi, nhi, d_out,
                        single_packet=False,
                    )
                gqt = gq.tile([P, cap, P], BF16, tag="gqt")
                if STRIP_LEVEL == 6:
                    nc.gpsimd.memset(gqt[:], 0.0)
                else:
                    nc.gpsimd.dma_gather(
                        gqt[:, :, :], q_pad[:, :],
                        qi_sb[:, i0 : i0 + wcol], nq, nq, P,
                        single_packet=False,
                    )

                if STRIP_LEVEL >= 3:
                    ot = op.tile([P, d_out], F32, tag="ot")
                    nc.vector.tensor_copy(ot[:], g[:, 0, :])
                    nc.sync.dma_start(
                        out=out_d[n0 : n0 + rows, :], in_=ot[0:rows, :]
                    )
                    continue

                # s = <z_row, a_src> per edge: bcast-mult + reduce
                sv = wp.tile([P, cap], F32, tag="sv")
                prod = wp.tile([P, cap, P], BF16, tag="prod")
                nc.vector.tensor_tensor(
                    out=prod[:, :, :],
                    in0=g[:, :, :],
                    in1=arep3[:, :, :].to_broadcast([P, cap, P]),
                    op=mybir.AluOpType.mult,
                )
                nc.vector.tensor_reduce(
                    out=sv[:],
                    in_=prod[:, :, :],
                    axis=mybir.AxisListType.X,
                    op=mybir.AluOpType.add,
                )
                qv = wp.tile([P, cap], F32, tag="qv")
                nc.vector.tensor_tensor(
                    out=qv[:],
                    in0=gqt[:, :, 0],
                    in1=gqt[:, :, 1],
                    op=mybir.AluOpType.add,
                )
                x = wp.tile([P, cap], F32, tag="x")
                nc.vector.tensor_tensor(
                    out=x[:], in0=sv[:], in1=qv[:], op=mybir.AluOpType.add
                )
                x2 = wp.tile([P, cap], F32, tag="x2")
                nc.scalar.activation(
                    out=x2[:], in_=x[:],
                    func=mybir.ActivationFunctionType.Copy, scale=0.01,
                )
                xm = wp.tile([P, cap], F32, tag="xm")
                nc.vector.tensor_tensor(
                    out=xm[:], in0=x[:], in1=x2[:], op=mybir.AluOpType.max
                )
                ex = wp.tile([P, cap], F32, tag="ex")
                nc.scalar.activation(
                    out=ex[:], in_=xm[:], func=mybir.ActivationFunctionType.Exp
                )

                if STRIP_LEVEL >= 2:
                    ot = op.tile([P, d_out], F32, tag="ot")
                    nc.scalar.activation(
                        out=ot[:], in_=g[:, 0, :],
                        func=mybir.ActivationFunctionType.Copy, scale=ex[:, 0:1],
                    )
                    nc.sync.dma_start(
                        out=out_d[n0 : n0 + rows, :], in_=ot[0:rows, :]
                    )
                    continue

                po = psum2.tile([P, d_out], F32, tag="po")
                pod = psum2.tile([P, 1], F32, tag="pod")
                for c in range(cap):
                    s_sel = sp.tile([P, P], BF16, tag="s_sel")
                    nc.vector.tensor_scalar(
                        out=s_sel[:],
                        in0=iota_bf[:],
                        scalar1=seg_sb[:, c0 + c : c0 + c + 1],
                        scalar2=ex[:, c : c + 1],
                        op0=mybir.AluOpType.is_equal,
                        op1=mybir.AluOpType.mult,
                    )
                    nc.tensor.matmul(
                        out=po[:],
                        lhsT=s_sel[:],
                        rhs=g[:, c, :],
                        start=(c == 0),
                        stop=(c == cap - 1),
                    )
                    nc.tensor.matmul(
                        out=pod[:, 0:1],
                        lhsT=s_sel[:],
                        rhs=ones_col[:],
                        start=(c == 0),
                        stop=(c == cap - 1),
                    )

                den = wp.tile([P, 1], F32, tag="den")
                nc.scalar.activation(
                    out=den[:], in_=pod[:, 0:1],
                    func=mybir.ActivationFunctionType.Copy, bias=1e-6,
                )
                rec = wp.tile([P, 1], F32, tag="rec")
                nc.vector.reciprocal(rec[:], den[:])
                ot = op.tile([P, d_out], F32, tag="ot")
                nc.scalar.activation(
                    out=ot[:], in_=po[:, 0:d_out],
                    func=mybir.ActivationFunctionType.Copy, scale=rec[:, 0:1],
                )
                if w % 2 == 0:
                    nc.sync.dma_start(
                        out=out_d[n0 : n0 + rows, :], in_=ot[0:rows, :]
                    )
                else:
                    nc.scalar.dma_start(
                        out=out_d[n0 : n0 + rows, :], in_=ot[0:rows, :]
                    )

    nc.compile()
    return nc


# ---------------------------------------------------------------- driver
def prepare(h, W, a, src, dst):
    """Build (cached) program + per-core in_maps from full inputs."""
    h = np.asarray(h, dtype=np.float32)
    W = np.asarray(W, dtype=np.float32)
    a = np.asarray(a, dtype=np.float32)
    n_nodes = h.shape[0]
    shard = n_nodes // CORES

    capl, caph, arrs = preprocess_indices(src, dst, n_nodes=n_nodes)
    key = (capl, caph, n_nodes, h.shape[1], W.shape[0], STRIP_LEVEL, LIMIT_WINDOWS)
    if key not in _PROGRAM_CACHE:
        _PROGRAM_CACHE[key] = build_program(
            capl, caph, n_nodes=n_nodes, d_in=h.shape[1], d_out=W.shape[0]
        )
    nc = _PROGRAM_CACHE[key]

    in_maps = []
    for c in range(CORES):
        h_t_c = np.ascontiguousarray(h[c * shard : (c + 1) * shard].T)
        arep = np.ascontiguousarray(
            np.broadcast_to(a[: W.shape[0], 0][None, :], (P, W.shape[0]))
        ).astype(np.float32)
        m = {"h_t": h_t_c, "W": W, "a": a, "arep": arep}
        m.update(arrs[c])
        in_maps.append(m)
    return nc, in_maps


def kernel(h, W, a, src, dst):
    from concourse.bass_utils import run_bass_kernel_spmd

    nc, in_maps = prepare(h, W, a, src, dst)
    res = run_bass_kernel_spmd(nc, in_maps, core_ids=list(range(CORES)))
    outs = [res.results[c]["out"] for c in range(CORES)]
    return np.ascontiguousarray(np.concatenate(outs, axis=0).astype(np.float32))


# revision 23
# speedup vs baseline: 1.2084x; 1.2084x over previous
"""GAT node-level layer on 8 TRN2 NeuronCores.

Strategy: destination-sharded edge processing, bulk dma_gather for z,
descriptor-free on-chip path for the dst-side logits.
 - Host: sort edges by dst, shard by dst range (6250 nodes/core),
   window = 128 consecutive dst, chunk = 128 edges.  Within each window
   edges split into lo (src < 32768) / hi chunk groups (dma_gather
   int16 indices).  Padding slots gather row 0 and die via seg=-1.
 - Device phase 1: z = h @ W.T, s = z@a_src, q = z@a_dst per node.
   Packed 512B z_tab row: [z(0:128) | 1 | s_hi | s_lo | pad...] so the
   softmax-denominator ones column and the src logit ride the gather.
   AllGather compact [shard,132] -> strided z_tab[:, 0:132] rows.
 - Device phase 2 per window: two dma_gathers (lo/hi) fetch cap*128
   packed rows; per chunk a 0/1 dst mask (fast single-op tensor_scalar)
   is PE-transposed and multiplied with the window q column to expand
   q per edge (no per-edge q DMA); e = leaky_relu(s+q), exp without
   max-subtraction (shift cancels in softmax); S = mask * exp_e via ACT
   per-partition scale; PE accumulates out[d,0:129] += S_c.T @ [z|1]
   (col 128 = denominator); normalize; write out.
"""

import sys

if "/opt/trn_rl_repo" not in sys.path:
    sys.path.insert(0, "/opt/trn_rl_repo")

from contextlib import ExitStack

import numpy as np

from concourse import bacc, bass, mybir, tile
from concourse.masks import make_identity

N_NODES = 50000
N_EDGES = 800000
D_IN = 256
D_OUT = 128
CORES = 8
P = 128
SPLIT = 32768  # int16 idx limit for dma_gather
ZROW = 256  # padded z_tab row (512B)
ZW = 132  # used columns: z(128) | 1 | s_hi | s_lo

F32 = mybir.dt.float32
BF16 = mybir.dt.bfloat16
I16 = mybir.dt.int16

_PROGRAM_CACHE: dict = {}


# ---------------------------------------------------------------- host prep
def preprocess_indices(src, dst, n_nodes=N_NODES, cores=CORES):
    """Sort edges by dst, shard by dst range, build per-core padded
    dma_gather index arrays (int16, [16,S]-wrapped, replicated to 128
    partitions) and seg masks."""
    shard = n_nodes // cores
    wpc = (shard + P - 1) // P
    src = np.asarray(src).astype(np.int64)
    dst = np.asarray(dst).astype(np.int64)

    order = np.argsort(dst, kind="stable")
    ds = dst[order]
    ss = src[order]
    bounds = np.searchsorted(ds, np.arange(cores + 1) * shard)

    per_core = []
    capl = caph = 1
    for c in range(cores):
        lo_, hi_ = int(bounds[c]), int(bounds[c + 1])
        dloc = ds[lo_:hi_] - c * shard
        s_c = ss[lo_:hi_]
        w = dloc >> 7
        is_hi = s_c >= SPLIT
        nlo = np.bincount(w[~is_hi], minlength=wpc)
        nhi = np.bincount(w[is_hi], minlength=wpc)
        capl = max(capl, int((nlo.max() + P - 1) // P))
        caph = max(caph, int((nhi.max() + P - 1) // P))
        per_core.append((dloc, s_c, w, is_hi))

    cap = capl + caph
    arrs = []
    for c in range(cores):
        dloc, s_c, w, is_hi = per_core[c]
        slot = np.zeros(len(dloc), np.int64)
        for grp, base in ((~is_hi, 0), (is_hi, capl * P)):
            sel = np.flatnonzero(grp)
            if len(sel) == 0:
                continue
            ww = w[sel]
            o = np.argsort(ww, kind="stable")
            sel = sel[o]
            ww = w[sel]
            cnt = np.bincount(ww, minlength=wpc)
            st = np.zeros(wpc, np.int64)
            st[1:] = np.cumsum(cnt)[:-1]
            pos = np.arange(len(sel), dtype=np.int64) - st[ww]
            slot[sel] = base + pos

        chunk = slot >> 7
        epos = slot & 127

        flat = w * (cap * P) + chunk * P + epos
        zi_flat = np.zeros(wpc * cap * P, np.int16)
        zval = np.where(is_hi, s_c - SPLIT, s_c).astype(np.int16)
        zi_flat[flat] = zval
        zi = zi_flat.reshape(wpc, cap * P)

        seg = np.full((P, wpc * cap), -1.0, np.float32)
        seg[epos, w * cap + chunk] = (dloc & 127).astype(np.float32)

        # wrap to [16, S] (flat j at [j%16, j//16]), replicate to 128 parts
        out16 = np.zeros((16, wpc, cap * P // 16), np.int16)
        j = np.arange(cap * P)
        out16[j % 16, :, j // 16] = zi.T[j]
        zi16 = np.ascontiguousarray(
            np.tile(out16.reshape(16, wpc * cap * P // 16), (8, 1))
        )
        arrs.append({"zi16": zi16, "seg_all": np.ascontiguousarray(seg)})
    return capl, caph, arrs


# ---------------------------------------------------------------- program
def build_program(capl, caph, n_nodes=N_NODES, d_in=D_IN, d_out=D_OUT, cores=CORES):
    shard = n_nodes // cores
    wpc = (shard + P - 1) // P
    kc_n = d_in // P
    cap = capl + caph
    nlo, nhi = capl * P, caph * P
    wcol = cap * P // 16

    nc = bacc.Bacc(None, target_bir_lowering=False, debug=False)

    h_t = nc.dram_tensor("h_t", [d_in, shard], F32, kind="ExternalInput")
    w_d = nc.dram_tensor("W", [d_out, d_in], F32, kind="ExternalInput")
    a_d = nc.dram_tensor("a", [2 * d_out, 1], F32, kind="ExternalInput")
    zi_d = nc.dram_tensor("zi16", [P, wpc * wcol], I16, kind="ExternalInput")
    seg_d = nc.dram_tensor("seg_all", [P, wpc * cap], F32, kind="ExternalInput")
    out_d = nc.dram_tensor("out", [shard, d_out], F32, kind="ExternalOutput")

    rg = [list(range(cores))]

    with tile.TileContext(nc) as tc:
        with ExitStack() as ctx:
            dram = ctx.enter_context(tc.tile_pool(name="dram", bufs=1, space="DRAM"))
            z_bounce = dram.tile([shard, ZW], BF16)
            z_tab = dram.tile([n_nodes, ZROW], BF16)
            q_hl = dram.tile([shard, 2], BF16)

            const = ctx.enter_context(tc.tile_pool(name="const", bufs=1))

            # ---- constants
            identity = const.tile([P, P], F32)
            make_identity(nc, identity[:])
            identity_bf = const.tile([P, P], BF16)
            nc.vector.tensor_copy(identity_bf[:], identity[:])
            iota_i = const.tile([P, P], mybir.dt.int32)
            nc.gpsimd.iota(iota_i[:], pattern=[[1, P]], base=0, channel_multiplier=0)
            iota_bf = const.tile([P, P], BF16)
            nc.vector.tensor_copy(iota_bf[:], iota_i[:])
            ones_col = const.tile([P, 1], BF16)
            nc.gpsimd.memset(ones_col[:], 1.0)

            zi_sb = const.tile([P, wpc * wcol], I16)
            seg_sb = const.tile([P, wpc * cap], F32)
            nc.sync.dma_start(out=zi_sb[:], in_=zi_d[:, :])
            nc.sync.dma_start(out=seg_sb[:], in_=seg_d[:, :])

            w_sb = const.tile([P, d_in], F32)
            nc.sync.dma_start(out=w_sb[:], in_=w_d[:, :])
            a_sb = const.tile([P, 2], F32)
            nc.sync.dma_start(out=a_sb[:, 0:1], in_=a_d[0:P, :])
            nc.sync.dma_start(out=a_sb[:, 1:2], in_=a_d[P : 2 * P, :])

            # waug[kc] = [W.T chunk | v_src | v_dst]  (bf16)
            ctx1 = ctx.enter_context(ExitStack())
            psum = ctx1.enter_context(tc.tile_pool(name="psum", bufs=1, space="PSUM"))
            psum_pz = ctx1.enter_context(
                tc.tile_pool(name="psum_pz", bufs=2, space="PSUM")
            )
            waug = const.tile([P, kc_n, d_out + 2], BF16)
            for kc in range(kc_n):
                ksl = slice(kc * P, (kc + 1) * P)
                pt = psum.tile([P, P], F32, tag="pt")
                nc.tensor.transpose(pt[:], w_sb[:, ksl], identity[:])
                nc.vector.tensor_copy(waug[:, kc, 0:d_out], pt[:])
                pv = psum.tile([P, 2], F32, tag="pv")
                nc.tensor.matmul(
                    out=pv[:, 0:1], lhsT=w_sb[:, ksl], rhs=a_sb[:, 0:1],
                    start=True, stop=True,
                )
                nc.tensor.matmul(
                    out=pv[:, 1:2], lhsT=w_sb[:, ksl], rhs=a_sb[:, 1:2],
                    start=True, stop=True,
                )
                nc.vector.tensor_copy(waug[:, kc, d_out : d_out + 2], pv[:])

            # ---- phase 1: z/s/q per node, pack rows
            h_sb = const.tile([P, kc_n, shard], BF16)
            for kc in range(kc_n):
                nc.gpsimd.dma_start(
                    out=h_sb[:, kc, :], in_=h_t[kc * P : (kc + 1) * P, :]
                )

            zq = ctx.enter_context(tc.tile_pool(name="zq", bufs=3))
            for nt in range(wpc):
                n0 = nt * P
                rows = min(P, shard - n0)
                pz = psum_pz.tile([P, d_out + 2], F32, tag="pz")
                for kc in range(kc_n):
                    nc.tensor.matmul(
                        out=pz[0:rows, :],
                        lhsT=h_sb[:, kc, n0 : n0 + rows],
                        rhs=waug[:, kc, :],
                        start=(kc == 0),
                        stop=(kc == kc_n - 1),
                    )
                # packed row: [z(0:128) | 1(128) | s_hi(129) | s_lo(130) | pad]
                zt = zq.tile([P, ZW], BF16, tag="zt")
                nc.vector.tensor_copy(zt[0:rows, 0:d_out], pz[0:rows, 0:d_out])
                nc.vector.tensor_copy(
                    zt[0:rows, d_out : d_out + 1], ones_col[0:rows, :]
                )
                nc.vector.tensor_copy(
                    zt[0:rows, d_out + 1 : d_out + 2], pz[0:rows, d_out : d_out + 1]
                )
                nc.vector.tensor_tensor(
                    out=zt[0:rows, d_out + 2 : d_out + 3],
                    in0=pz[0:rows, d_out : d_out + 1],
                    in1=zt[0:rows, d_out + 1 : d_out + 2],
                    op=mybir.AluOpType.subtract,
                )
                qhl = zq.tile([P, 2], BF16, tag="qhl")
                nc.vector.tensor_copy(
                    qhl[0:rows, 0:1], pz[0:rows, d_out + 1 : d_out + 2]
                )
                nc.vector.tensor_tensor(
                    out=qhl[0:rows, 1:2],
                    in0=pz[0:rows, d_out + 1 : d_out + 2],
                    in1=qhl[0:rows, 0:1],
                    op=mybir.AluOpType.subtract,
                )
                if nt % 2 == 0:
                    nc.sync.dma_start(
                        out=z_bounce[n0 : n0 + rows, :], in_=zt[0:rows, :]
                    )
                else:
                    nc.scalar.dma_start(
                        out=z_bounce[n0 : n0 + rows, :], in_=zt[0:rows, :]
                    )
                nc.sync.dma_start(out=q_hl[n0 : n0 + rows, :], in_=qhl[0:rows, :])

            ctx1.close()

            # AllGather compact [shard,132], then expand to 512B z_tab rows
            z_fullc = dram.tile([n_nodes, ZW], BF16)
            nc.gpsimd.collective_compute(
                "AllGather",
                mybir.AluOpType.bypass,
                replica_groups=rg,
                ins=[z_bounce[:, :]],
                outs=[z_fullc[:, :]],
            )
            nexp = 8
            estep = n_nodes // nexp
            for e in range(nexp):
                lo, hi = e * estep, min((e + 1) * estep, n_nodes)
                eng = nc.sync if e % 2 == 0 else nc.scalar
                eng.dma_start(
                    out=z_tab[lo:hi, 0:ZW], in_=z_fullc[lo:hi, :]
                )

            # ---- phase 2: per dst-window edge processing
            gp = ctx.enter_context(tc.tile_pool(name="gp", bufs=2))
            mp = ctx.enter_context(tc.tile_pool(name="mp", bufs=2))
            wp = ctx.enter_context(tc.tile_pool(name="wp", bufs=3))
            sp = ctx.enter_context(tc.tile_pool(name="sp", bufs=4))
            op = ctx.enter_context(tc.tile_pool(name="op", bufs=3))
            ps_mt = ctx.enter_context(
                tc.tile_pool(name="ps_mt", bufs=2, space="PSUM")
            )
            ps_qv = ctx.enter_context(
                tc.tile_pool(name="ps_qv", bufs=2, space="PSUM")
            )
            ps_po = ctx.enter_context(
                tc.tile_pool(name="ps_po", bufs=2, space="PSUM")
            )

            for w in range(wpc):
                n0 = w * P
                rows = min(P, shard - n0)
                c0 = w * cap
                i0 = w * wcol

                g = gp.tile([P, cap, ZROW], BF16, tag="g")
                nc.gpsimd.dma_gather(
                    g[:, 0:capl, :], z_tab[0:SPLIT, :],
                    zi_sb[:, i0 : i0 + nlo // 16], nlo, nlo, ZROW,
                    single_packet=False,
                )
                nc.gpsimd.dma_gather(
                    g[:, capl:cap, :], z_tab[SPLIT:n_nodes, :],
                    zi_sb[:, i0 + nlo // 16 : i0 + wcol], nhi, nhi, ZROW,
                    single_packet=False,
                )
                qhl_w = wp.tile([P, 2], BF16, tag="qhl_w")
                nc.sync.dma_start(out=qhl_w[0:rows, :], in_=q_hl[n0 : n0 + rows, :])

                # pass A: masks + q expansion via PE transpose
                masks = mp.tile([P, cap, P], BF16, tag="masks")
                qv2 = ps_qv.tile([P, cap, 2], F32, tag="qv2")
                for c in range(cap):
                    nc.vector.tensor_scalar(
                        out=masks[:, c, :],
                        in0=iota_bf[:],
                        scalar1=seg_sb[:, c0 + c : c0 + c + 1],
                        scalar2=None,
                        op0=mybir.AluOpType.is_equal,
                    )
                    mt_ps = ps_mt.tile([P, P], BF16, tag="mt")
                    nc.tensor.transpose(mt_ps[:], masks[:, c, :], identity_bf[:])
                    mt_sb = sp.tile([P, P], BF16, tag="mt_sb")
                    nc.vector.tensor_copy(mt_sb[:], mt_ps[:])
                    nc.tensor.matmul(
                        out=qv2[:, c, :], lhsT=mt_sb[:], rhs=qhl_w[:, :],
                        start=True, stop=True,
                    )

                # logits: e = lrelu(s + q); ex = exp(e)
                sv = wp.tile([P, cap], F32, tag="sv")
                nc.vector.tensor_tensor(
                    out=sv[:],
                    in0=g[:, :, d_out + 1],
                    in1=g[:, :, d_out + 2],
                    op=mybir.AluOpType.add,
                )
                x1 = wp.tile([P, cap], F32, tag="x1")
                nc.vector.tensor_tensor(
                    out=x1[:], in0=sv[:], in1=qv2[:, :, 0], op=mybir.AluOpType.add
                )
                x = wp.tile([P, cap], F32, tag="x")
                nc.vector.tensor_tensor(
                    out=x[:], in0=x1[:], in1=qv2[:, :, 1], op=mybir.AluOpType.add
                )
                x2 = wp.tile([P, cap], F32, tag="x2")
                nc.scalar.activation(
                    out=x2[:], in_=x[:],
                    func=mybir.ActivationFunctionType.Copy, scale=0.01,
                )
                xm = wp.tile([P, cap], F32, tag="xm")
                nc.vector.tensor_tensor(
                    out=xm[:], in0=x[:], in1=x2[:], op=mybir.AluOpType.max
                )
                ex = wp.tile([P, cap], F32, tag="ex")
                nc.scalar.activation(
                    out=ex[:], in_=xm[:], func=mybir.ActivationFunctionType.Exp
                )

                # pass B: S = mask * ex (ACT scale), accumulate po
                po = ps_po.tile([P, d_out + 1], F32, tag="po")
                for c in range(cap):
                    s_sel = sp.tile([P, P], BF16, tag="s_sel")
                    nc.scalar.activation(
                        out=s_sel[:], in_=masks[:, c, :],
                        func=mybir.ActivationFunctionType.Copy,
                        scale=ex[:, c : c + 1],
                    )
                    nc.tensor.matmul(
                        out=po[:],
                        lhsT=s_sel[:],
                        rhs=g[:, c, 0 : d_out + 1],
                        start=(c == 0),
                        stop=(c == cap - 1),
                    )

                den = wp.tile([P, 1], F32, tag="den")
                nc.scalar.activation(
                    out=den[:], in_=po[:, d_out : d_out + 1],
                    func=mybir.ActivationFunctionType.Copy, bias=1e-6,
                )
                rec = wp.tile([P, 1], F32, tag="rec")
                nc.vector.reciprocal(rec[:], den[:])
                ot = op.tile([P, d_out], F32, tag="ot")
                nc.scalar.activation(
                    out=ot[:], in_=po[:, 0:d_out],
                    func=mybir.ActivationFunctionType.Copy, scale=rec[:, 0:1],
                )
                if w % 2 == 0:
                    nc.sync.dma_start(
                        out=out_d[n0 : n0 + rows, :], in_=ot[0:rows, :]
                    )
                else:
                    nc.scalar.dma_start(
                        out=out_d[n0 : n0 + rows, :], in_=ot[0:rows, :]
                    )

    nc.compile()
    return nc


# ---------------------------------------------------------------- driver
def prepare(h, W, a, src, dst):
    """Build (cached) program + per-core in_maps from full inputs."""
    h = np.asarray(h, dtype=np.float32)
    W = np.asarray(W, dtype=np.float32)
    a = np.asarray(a, dtype=np.float32)
    n_nodes = h.shape[0]
    shard = n_nodes // CORES

    capl, caph, arrs = preprocess_indices(src, dst, n_nodes=n_nodes)
    key = (capl, caph, n_nodes, h.shape[1], W.shape[0])
    if key not in _PROGRAM_CACHE:
        _PROGRAM_CACHE[key] = build_program(
            capl, caph, n_nodes=n_nodes, d_in=h.shape[1], d_out=W.shape[0]
        )
    nc = _PROGRAM_CACHE[key]

    in_maps = []
    for c in range(CORES):
        h_t_c = np.ascontiguousarray(h[c * shard : (c + 1) * shard].T)
        m = {"h_t": h_t_c, "W": W, "a": a}
        m.update(arrs[c])
        in_maps.append(m)
    return nc, in_maps


def kernel(h, W, a, src, dst):
    from concourse.bass_utils import run_bass_kernel_spmd

    nc, in_maps = prepare(h, W, a, src, dst)
    res = run_bass_kernel_spmd(nc, in_maps, core_ids=list(range(CORES)))
    outs = [res.results[c]["out"] for c in range(CORES)]
    return np.ascontiguousarray(np.concatenate(outs, axis=0).astype(np.float32))
